# revision 1
# baseline (speedup 1.0000x reference)
"""CRF loss (partition - score) Trainium2 kernel.

Problem: B=512, S=1024, T=48 CRF forward algorithm (log-partition via
sequential logsumexp recursion), data-parallel over 8 NeuronCores (64
batch elements per core).

Algorithm (per core, all in probability space):
  - Work with u_t = exp(alpha_t), so the per-step logsumexp becomes a tiny
    matmul against E = exp(transitions) plus an elementwise multiply by
    w_t = exp(emissions_t):
        fwd:  a_t[j] = w_t[j] * sum_i E[i,j] a_{t-1}[i]
        bwd:  g_t[i] = w_t[i] * sum_j E[i,j] g_{t+1}[j]
  - Meet-in-the-middle: forward chain from t=0 and backward chain from
    t=S-1 are independent; Z = a_{K-1}^T E g_K with K = S/2.  Both chains
    are stacked on partitions 0..95 of the same tiles, so one matmul
    (block-diagonal stationary) + one VectorE multiply advances both.
  - The batch is split into CHAINS interleaved column groups so the PE
    matmul of one group overlaps the VectorE multiply of the other
    (the recurrence itself is serial per group).
  - State and stationaries are bf16 (single-pass matmuls; fp32 matmuls
    lower to two PE passes).  PSUM accumulation stays fp32.
  - E is pre-scaled by exp(-c0) (c0 = average per-step log-growth,
    calibrated on the host with a tiny float64 sim) so state magnitude
    drifts only as a random walk.  Every RENORM steps a chain is rescaled
    by an exact power of two: s = column sums (matmul), bf16(s) stored to
    a log tile, and the scale 2^(127-e) is built with one VectorE integer
    op ((bits & 0x7F80) ^ 0x7F80 on the bf16 exponent, halved via a 0.5
    broadcast matmul) — no ScalarE in the loop, no rounding of the state.
    The host recovers the exact applied scales from the stored bf16 bits.
  - Emissions are restaged on the host into the exact [96, K, BL] layout
    each core consumes, so every DMA chunk is a single fully-contiguous
    transfer; exp() runs on ScalarE in bulk, off the critical path.

The reference computes `partition - score` where both are the identical
forward algorithm when the mask is all ones (the spec pins mask to ones);
the masked recursion's where(mask, new, old) is the identity then, so
score == partition bitwise.  The kernel computes the shared forward pass
on device and returns their difference.  A faithful numpy fallback
handles a non-all-ones mask, should one ever be passed.
"""

import ml_dtypes
import numpy as np

import concourse.bass as bass
import concourse.bacc as bacc
import concourse.tile as tile
import concourse.mybir as mybir
from concourse.bass_utils import run_bass_kernel_spmd

F32 = mybir.dt.float32
BF16 = mybir.dt.bfloat16
U16 = mybir.dt.uint16
AFT = mybir.ActivationFunctionType
ALU = mybir.AluOpType

N_CORES = 8
B, S, T = 512, 1024, 48
BL = B // N_CORES          # 64 batch elements per core
K = S // 2                 # 512 meta-steps (bidirectional)
CH = 32                    # (legacy; chunking now follows chunk_plan)
KC = K // CH               # meta-steps per chunk (legacy default)
P2 = 2 * T                 # 96 partitions: rows 0..47 fwd, 48..95 bwd
RENORM = 512               # renormalize every RENORM meta-steps (per chain)
NO_RELOAD = False          # ldweights=False measured neutral (LDW fully overlaps)
EXP_SPLIT = 1              # ScalarE exp instructions per chunk
CHAINS = 2                 # interleaved batch column groups
NRMAX = 16                 # sacc slots per chain

# module-level knobs / results (test.py uses these)
TRACE = False
LAST_RESULTS = None

_program_cache = {}


def chunk_plan(K, KC=None):
    """Graded chunk sizes: small first chunks for a fast pipeline ramp,
    64-step chunks afterwards for few tile transitions."""
    if KC is not None:                      # explicit uniform chunking
        return [(k, KC) for k in range(0, K, KC)]
    plan, k = [], 0
    for size in [8, 8, 16, 32]:
        size = min(size, K - k)
        if size > 0:
            plan.append((k, size))
            k += size
    while k < K:
        size = min(64, K - k)
        plan.append((k, size))
        k += size
    return plan


def renorm_steps(K, renorm, chains, g):
    """Meta-steps at which chain g renormalizes (phase-split across chains)."""
    phase = (g * renorm) // chains
    return [k for k in range(1, K)
            if k % renorm == phase and k >= renorm // chains]


def build_program(P2=P2, BL=BL, K=K, CH=CH, KC=KC, renorm=RENORM,
                  exp_split=EXP_SPLIT, chains=CHAINS, num_devices=N_CORES):
    """Build + compile the per-core Bass/Tile program (SPMD, no collectives)."""
    Tn = P2 // 2
    CW = 96 + 2 + Tn + 2 + 96  # consts cols: blockE | sum | fin | ones(pad) | bc
    CB = BL // chains          # batch columns per chain
    SW = chains * NRMAX * CB   # sacc columns
    nc = bacc.Bacc(
        "TRN2",
        target_bir_lowering=False,
        debug=False,
        num_devices=num_devices,
    )
    wstg = nc.dram_tensor("wstg", [P2, K, BL], F32, kind="ExternalInput").ap()
    consts = nc.dram_tensor("consts", [P2, CW], BF16, kind="ExternalInput").ap()
    out_z = nc.dram_tensor("zraw", [1, BL], F32, kind="ExternalOutput").ap()
    out_s = nc.dram_tensor("sacc", [2, SW], BF16, kind="ExternalOutput").ap()

    rsteps = {g: set(renorm_steps(K, renorm, chains, g)) for g in range(chains)}
    rindex = {g: {k: i for i, k in enumerate(sorted(rsteps[g]))}
              for g in range(chains)}

    with tile.TileContext(nc) as tc:
        with (
            tc.tile_pool(name="consts", bufs=1) as cpool,
            tc.tile_pool(name="raw", bufs=2) as rawpool,
            tc.tile_pool(name="wexp", bufs=2) as wpool,
            tc.tile_pool(name="state", bufs=2) as xpool,
            tc.tile_pool(name="sacc_p", bufs=1) as sapool,
            tc.tile_pool(name="small", bufs=2) as smpool,
            tc.tile_pool(name="psum_v", bufs=2, space=bass.MemorySpace.PSUM) as ppool,
            tc.tile_pool(name="psum_r", bufs=1, space=bass.MemorySpace.PSUM) as ppool_r,
            tc.tile_pool(name="psum_f", bufs=1, space=bass.MemorySpace.PSUM) as ppool_f,
        ):
            # first emission chunk DMA is issued before anything else so the
            # scan pipeline ramps as early as possible; consts follow on the
            # same ring and still land long before the first matmul.
            plan = chunk_plan(K) if (CH * KC == K and K == 512) else chunk_plan(K, KC)
            k0f, klenf = plan[0]
            raw0 = rawpool.tile([P2, klenf * BL], F32, tag="raw", name="raw0")
            nc.sync.dma_start(
                raw0[:], wstg[:, k0f:k0f + klenf, :].rearrange("p k b -> p (k b)"))
            cst = cpool.tile([P2, CW], BF16)
            nc.sync.dma_start(cst[:], consts)
            blockE = cst[:, 0:96]
            lhsT_sum = cst[:, 96:98]
            lhsT_fin = cst[:, 98:98 + Tn]
            ones_col = cst[0:Tn, 98 + Tn:99 + Tn]
            lhsT_bc = cst[0:2, 100 + Tn:100 + Tn + 96]  # entries 0.5

            sacc = sapool.tile([2, SW], BF16)
            nc.vector.memset(sacc[:], 0.0)

            xs = [None] * chains
            for ci, (k0, klen) in enumerate(plan):
                if ci == 0:
                    raw = raw0
                else:
                    raw = rawpool.tile([P2, klen * BL], F32, tag="raw", name="raw")
                    nc.sync.dma_start(
                        raw[:], wstg[:, k0:k0 + klen, :].rearrange("p k b -> p (k b)"))
                w = wpool.tile([P2, klen * BL], F32, tag="w", name="w")
                nc.scalar.activation(w[:], raw[:], AFT.Exp)
                for kl in range(klen):
                    kglob = k0 + kl
                    for g in range(chains):
                        wk = w[:, kl * BL + g * CB:kl * BL + (g + 1) * CB]
                        if kglob == 0:
                            xs[g] = xpool.tile([P2, CB], BF16, tag=f"x{g}", name=f"x{g}")
                            nc.vector.tensor_copy(xs[g][:], wk)
                            continue
                        v = ppool.tile([P2, CB], F32, tag=f"v{g}")
                        mm = nc.tensor.matmul(v[:], blockE, xs[g][:], start=True, stop=True)
                        if NO_RELOAD and kglob > 1 and not rsteps[g]:
                            # every PE matmul in the scan shares the blockE
                            # stationary (renorms disabled), so skip the
                            # per-matmul weight reload; kglob==1 self-loads.
                            mm.ins.ldweights = False
                        xs[g] = xpool.tile([P2, CB], BF16, tag=f"x{g}", name=f"x{g}")
                        # (v * 1.0) * w via the TensorScalarPtr op family —
                        # measured faster than tensor_tensor for this shape
                        nc.vector.scalar_tensor_tensor(
                            xs[g][:], v[:], 1.0, wk, ALU.mult, ALU.mult)
                        if kglob in rsteps[g]:
                            ri = rindex[g][kglob]
                            col = (g * NRMAX + ri) * CB
                            s = ppool_r.tile([2, CB], F32, tag="s")
                            nc.tensor.matmul(s[:], lhsT_sum, xs[g][:], start=True, stop=True)
                            sl = sacc[:, col:col + CB]
                            nc.vector.tensor_copy(sl, s[:])
                            rinv = smpool.tile([2, CB], BF16, tag="rinv")
                            nc.vector.tensor_scalar(
                                rinv[:].bitcast(U16), sl.bitcast(U16),
                                0x7F80, 0x7F80,
                                ALU.bitwise_and, ALU.bitwise_xor,
                            )
                            bc = ppool_r.tile([P2, CB], F32, tag="bc")
                            nc.tensor.matmul(bc[:], lhsT_bc, rinv[:], start=True, stop=True)
                            xn = xpool.tile([P2, CB], BF16, tag=f"x{g}")
                            nc.vector.tensor_mul(xn[:], xs[g][:], bc[:])
                            xs[g] = xn

            # final combine per chain: Z = a^T E' g  (a = x[0:Tn])
            for g in range(chains):
                x = xs[g]
                vf = ppool_f.tile([Tn, CB], F32, tag="vf")
                nc.tensor.matmul(vf[:], lhsT_fin, x[:], start=True, stop=True)
                tmp = smpool.tile([Tn, CB], BF16, tag="tmp")
                nc.vector.tensor_mul(tmp[:], vf[:], x[0:Tn, :])
                z = ppool_f.tile([1, CB], F32, tag="z")
                nc.tensor.matmul(z[:], ones_col, tmp[:], start=True, stop=True)
                zsb = smpool.tile([1, CB], F32, tag="zsb")
                nc.vector.tensor_copy(zsb[:], z[:])
                nc.sync.dma_start(out_z[:, g * CB:(g + 1) * CB], zsb[:])
            nc.sync.dma_start(out_s, sacc[:])

    nc.compile()
    return nc


def _get_program():
    key = "full"
    if key not in _program_cache:
        _program_cache[key] = build_program()
    return _program_cache[key]


def _calibrate_c0(emissions, start, trans, n_batches=8):
    """Average per-step log growth of the forward recursion (float64)."""
    idx = np.linspace(0, emissions.shape[0] - 1, n_batches).astype(np.int64)
    E = np.exp(trans.astype(np.float64))
    u = np.exp(start.astype(np.float64))[None, :] * \
        np.exp(emissions[idx, 0].astype(np.float64))
    s = u.sum(axis=1, keepdims=True)
    u /= s
    tot = 0.0
    n = emissions.shape[1]
    for t in range(1, n):
        u = np.exp(emissions[idx, t].astype(np.float64)) * (u @ E)
        s = u.sum(axis=1, keepdims=True)
        u /= s
        tot += np.log(s).mean()
    return tot / (n - 1)


def make_consts(Ep_bf16, Tn=T):
    CW = 96 + 2 + Tn + 2 + 96
    P2l = 2 * Tn
    consts = np.zeros((P2l, CW), ml_dtypes.bfloat16)
    consts[:Tn, :Tn] = Ep_bf16                 # fwd block
    consts[Tn:, Tn:2 * Tn] = Ep_bf16.T         # bwd block
    consts[:Tn, 96] = 1.0                      # lhsT_sum col 0: fwd sum
    consts[Tn:, 97] = 1.0                      # lhsT_sum col 1: bwd sum
    consts[Tn:, 98:98 + Tn] = Ep_bf16.T        # lhsT_fin
    consts[:Tn, 98 + Tn] = 1.0                 # ones_col
    consts[0, 100 + Tn:100 + 2 * Tn] = 0.5     # lhsT_bc row 0 -> fwd rows
    consts[1, 100 + 2 * Tn:100 + Tn + 96] = 0.5  # lhsT_bc row 1 -> bwd rows
    return consts


def stage_inputs(emissions, start, end, trans):
    """Host-side restaging: per-core [P2, K, BL] emission tiles + consts."""
    c0 = _calibrate_c0(emissions, start, trans)
    Ep = (np.exp(trans.astype(np.float64)) * np.exp(-c0)).astype(ml_dtypes.bfloat16)
    consts = make_consts(Ep)

    in_maps = []
    for core in range(N_CORES):
        sl = slice(core * BL, (core + 1) * BL)
        stg = np.empty((P2, K, BL), np.float32)
        stg[:T] = emissions[sl, :K, :].transpose(2, 1, 0)
        stg[:T, 0, :] += start[:, None]
        stg[T:] = emissions[sl, K:, :][:, ::-1, :].transpose(2, 1, 0)
        stg[T:, 0, :] += end[:, None]
        in_maps.append({"wstg": stg, "consts": consts})
    return in_maps, c0


def unpack_logZ(zraw, sacc_bits, c0, K=K, renorm=RENORM, chains=CHAINS,
                BL=BL):
    """Recover logZ[BL] from device outputs of one core (float64 host math)."""
    CB = BL // chains
    n_scale = 2 * (K - 1) + 1
    logZ = np.log(zraw.astype(np.float64)) + n_scale * c0  # [BL]
    ln2 = np.log(2.0)
    for g in range(chains):
        nr = len(renorm_steps(K, renorm, chains, g))
        for ri in range(nr):
            col = (g * NRMAX + ri) * CB
            bits = sacc_bits[:, col:col + CB]  # uint16 [2, CB]
            e = ((bits >> 7) & 0xFF).astype(np.float64)
            # applied scale was 2^(127-e) per (half, batch); undo both halves
            logZ[g * CB:(g + 1) * CB] += ((e[0] - 127.0) + (e[1] - 127.0)) * ln2
    return logZ


def _device_logZ(emissions, start, end, trans):
    global LAST_RESULTS
    nc = _get_program()
    in_maps, c0 = stage_inputs(emissions, start, end, trans)
    res = run_bass_kernel_spmd(
        nc, in_maps, core_ids=list(range(N_CORES)), trace=TRACE,
    )
    LAST_RESULTS = res
    logZ = np.empty(B, np.float32)
    for core in range(N_CORES):
        r = res.results[core]
        zraw = r["zraw"][0]
        sacc = np.asarray(r["sacc"]).view(np.uint16)
        logZ[core * BL:(core + 1) * BL] = unpack_logZ(zraw, sacc, c0).astype(np.float32)
    return logZ


def _numpy_fallback(emissions, mask, start, end, trans):
    """Faithful float64 reference implementation (handles any mask)."""
    def fwd(use_mask):
        a = start[None, :].astype(np.float64) + emissions[:, 0].astype(np.float64)
        tr = trans.astype(np.float64)
        for t in range(1, emissions.shape[1]):
            inner = a[:, :, None] + tr[None] + emissions[:, t].astype(np.float64)[:, None, :]
            m = inner.max(axis=1, keepdims=True)
            new = np.log(np.exp(inner - m).sum(axis=1)) + m[:, 0, :]
            if use_mask:
                a = np.where(mask[:, t][:, None], new, a)
            else:
                a = new
        fin = a + end[None].astype(np.float64)
        m = fin.max(axis=1, keepdims=True)
        return np.log(np.exp(fin - m).sum(axis=1)) + m[:, 0]

    score = fwd(True)
    partition = fwd(False)
    return (partition - score).astype(np.float32)


def kernel(emissions, mask, start_transitions, end_transitions, transitions):
    emissions = np.asarray(emissions, dtype=np.float32)
    mask = np.asarray(mask)
    start = np.asarray(start_transitions, dtype=np.float32)
    end = np.asarray(end_transitions, dtype=np.float32)
    trans = np.asarray(transitions, dtype=np.float32)

    if not mask.all():
        return _numpy_fallback(emissions, mask, start, end, trans)

    # With an all-ones mask the masked recursion's where(mask, new, old) is
    # the identity, so score == partition; both come from the same forward
    # pass, computed on the 8 NeuronCores.
    logZ = _device_logZ(emissions, start, end, trans)
    partition = logZ
    score = logZ
    return (partition - score).astype(np.float32)



# revision 2
# speedup vs baseline: 2.8862x; 2.8862x over previous
"""CRF loss (partition - score) Trainium2 kernel — segmented-probe scan.

Problem: B=512, S=1024, T=48 CRF forward algorithm (log-partition via
sequential logsumexp recursion), data-parallel over 8 NeuronCores (64
batch elements per core).

Algorithm (per core, all in probability space):
  - Work with u_t = exp(alpha_t): the per-step logsumexp becomes a matmul
    against E = exp(transitions) plus an elementwise multiply by
    w_t = exp(emissions_t):  a_t[j] = w_t[j] * sum_i E[i,j] a_{t-1}[i].
  - The 1024-step recursion is split into R=16 segments of L=63 steps.
    Products of positive matrices forget their initial direction at a
    geometric rate (Birkhoff contraction), so each segment's chain is
    started W=16 steps early from a probe vector (the staged w at the
    warmup start); by the segment boundary its direction matches the true
    forward chain to ~1e-6.  Per segment the device records the column
    sums (1^T x) at local steps W-1 and N-1; the host stitches
      logZ = log n_end(1) + sum_{j=2..R-1} [log n_end(j) - log n_start(j)]
           + log(f . x_R(end)) - log n_start(R)
    (f = exp(end_transitions)), adding back k*c0 per record since E is
    pre-scaled by exp(-c0) (c0 = average per-step log growth, calibrated
    on the host in float64).
  - Two segments are stacked on the 96 partitions of one tile-chain
    (block-diagonal stationary), so 8 tile-chains of N=79 steps cover all
    16 segments.  Chains are grouped 4-wide: one [96x96]x[96,256] matmul
    advances a whole group, and ONE VectorE multiply (FD=256) applies the
    emissions for 4 chains — amortizing the DVE's ~120-cycle fixed PSUM
    read cost, which is the true bottleneck of this recursion (the mult
    must run on DVE: GPSIMD has no PSUM port, ScalarE has no
    tensor*tensor).  Two groups ping-pong so the DVE stays saturated
    while PE/DMA hide underneath.
  - exp() is precomputed on the HOST and staged as bf16 (halving DMA
    bytes and keeping ScalarE out of the pipeline entirely).  State and
    stationaries are bf16; PSUM accumulation is fp32.  No renorms are
    needed: 79-step chains drift only as a tiny random walk around the
    exp(-c0) prescale.

The reference computes `partition - score` where both are the identical
forward algorithm when the mask is all ones (the spec pins mask to ones);
the masked recursion's where(mask, new, old) is the identity then, so
score == partition bitwise.  The kernel computes the shared forward pass
on device and returns their difference.  A faithful numpy fallback
handles a non-all-ones mask, should one ever be passed.
"""

import ml_dtypes
import numpy as np

import concourse.bass as bass
import concourse.bacc as bacc
import concourse.tile as tile
import concourse.mybir as mybir
from concourse.bass_utils import run_bass_kernel_spmd

F32 = mybir.dt.float32
BF16 = mybir.dt.bfloat16
ALU = mybir.AluOpType

N_CORES = 8
B, S, T = 512, 1024, 48
BL = B // N_CORES          # 64 batch elements per core
P2 = 2 * T                 # 96 partitions: two segments stacked per chain

R = 16                     # segments
L = 63                     # real steps per segment (R*L + W == S)
W = 16                     # warmup steps (probe direction convergence)
N = L + W                  # 79 local steps per chain
NCH = R // 2               # 8 tile-chains
NG = 2                     # chain groups (ping-pong for DVE saturation)
GW = NCH // NG             # 4 chains per group
GF = GW * BL               # 256 free-dim columns per group op
K_REC = (W - 1, N - 1)     # record column sums at these local steps

# module-level knobs / results (test.py uses these)
TRACE = False
LAST_RESULTS = None

_program_cache = {}


def chunk_plan(n=N):
    """Graded chunk sizes: small first chunks for a fast pipeline ramp."""
    plan, k = [], 0
    for size in [3, 5, 8]:
        size = min(size, n - k)
        if size > 0:
            plan.append((k, size))
            k += size
    while k < n:
        size = min(16, n - k)
        plan.append((k, size))
        k += size
    return plan


def build_program(num_devices=N_CORES):
    """Build + compile the per-core Bass/Tile program (SPMD, no collectives)."""
    CW = 96 + 2 + 1            # consts cols: blockE | lhsT_sum | lhsT_fin
    SW = NG * len(K_REC) * GF  # sacc cols: (group, record) blocks of 256
    nc = bacc.Bacc(
        "TRN2",
        target_bir_lowering=False,
        debug=False,
        num_devices=num_devices,
    )
    wstg = nc.dram_tensor("wstg", [P2, N, NG * GF], BF16, kind="ExternalInput").ap()
    consts = nc.dram_tensor("consts", [P2, CW], BF16, kind="ExternalInput").ap()
    out_s = nc.dram_tensor("sacc", [2, SW], F32, kind="ExternalOutput").ap()
    out_z = nc.dram_tensor("zfin", [1, BL], F32, kind="ExternalOutput").ap()

    SW_STEP = NG * GF          # cols per step in wstg

    with tile.TileContext(nc) as tc:
        with (
            tc.tile_pool(name="consts", bufs=1) as cpool,
            tc.tile_pool(name="raw", bufs=2) as rawpool,
            tc.tile_pool(name="state", bufs=2) as xpool,
            tc.tile_pool(name="sacc_p", bufs=1) as sapool,
            tc.tile_pool(name="small", bufs=2) as smpool,
            tc.tile_pool(name="psum_v", bufs=2, space=bass.MemorySpace.PSUM) as ppool,
            tc.tile_pool(name="psum_r", bufs=2, space=bass.MemorySpace.PSUM) as ppool_r,
        ):
            # first emission chunk DMA is issued before anything else so the
            # scan pipeline ramps as early as possible.
            plan = chunk_plan()
            k0f, klenf = plan[0]
            raw0 = rawpool.tile([P2, klenf * SW_STEP], BF16, tag="raw", name="raw0")
            nc.sync.dma_start(
                raw0[:], wstg[:, k0f:k0f + klenf, :].rearrange("p k b -> p (k b)"))
            cst = cpool.tile([P2, CW], BF16)
            nc.sync.dma_start(cst[:], consts)
            blockE = cst[:, 0:96]
            lhsT_sum = cst[:, 96:98]
            lhsT_fin = cst[:, 98:99]

            sacc = sapool.tile([2, SW], F32)

            xs = [None] * NG
            for ci, (k0, klen) in enumerate(plan):
                if ci == 0:
                    raw = raw0
                else:
                    raw = rawpool.tile([P2, klen * SW_STEP], BF16, tag="raw", name="raw")
                    nc.sync.dma_start(
                        raw[:], wstg[:, k0:k0 + klen, :].rearrange("p k b -> p (k b)"))
                for kl in range(klen):
                    k = k0 + kl
                    for g in range(NG):
                        wk = raw[:, kl * SW_STEP + g * GF:kl * SW_STEP + (g + 1) * GF]
                        if k == 0:
                            xs[g] = xpool.tile([P2, GF], BF16, tag=f"x{g}", name=f"x{g}")
                            nc.vector.tensor_copy(xs[g][:], wk)
                            continue
                        v = ppool.tile([P2, GF], F32, tag=f"v{g}")
                        nc.tensor.matmul(v[:], blockE, xs[g][:], start=True, stop=True)
                        xs[g] = xpool.tile([P2, GF], BF16, tag=f"x{g}", name=f"x{g}")
                        # (v * 1.0) * w via the TensorScalarPtr op family —
                        # measured faster than tensor_tensor for this shape
                        nc.vector.scalar_tensor_tensor(
                            xs[g][:], v[:], 1.0, wk, ALU.mult, ALU.mult)
                        if k in K_REC:
                            ri = K_REC.index(k)
                            s = ppool_r.tile([2, GF], F32, tag="s")
                            nc.tensor.matmul(s[:], lhsT_sum, xs[g][:], start=True, stop=True)
                            col = (g * len(K_REC) + ri) * GF
                            nc.vector.tensor_copy(sacc[:, col:col + GF], s[:])

            # final f-dot for the last segment (chain 7 = group 1, cols 192:256)
            z = ppool_r.tile([1, BL], F32, tag="z")
            nc.tensor.matmul(z[:], lhsT_fin, xs[NG - 1][:, (GW - 1) * BL:GF],
                             start=True, stop=True)
            zsb = smpool.tile([1, BL], F32, tag="zsb")
            nc.vector.tensor_copy(zsb[:], z[:])
            nc.sync.dma_start(out_z, zsb[:])
            nc.sync.dma_start(out_s, sacc[:])

    nc.compile()
    return nc


def _get_program():
    key = "full"
    if key not in _program_cache:
        _program_cache[key] = build_program()
    return _program_cache[key]


def _calibrate_c0(emissions, start, trans, n_batches=8):
    """Average per-step log growth of the forward recursion (float64)."""
    idx = np.linspace(0, emissions.shape[0] - 1, n_batches).astype(np.int64)
    E = np.exp(trans.astype(np.float64))
    u = np.exp(start.astype(np.float64))[None, :] * \
        np.exp(emissions[idx, 0].astype(np.float64))
    s = u.sum(axis=1, keepdims=True)
    u /= s
    tot = 0.0
    n = emissions.shape[1]
    for t in range(1, n):
        u = np.exp(emissions[idx, t].astype(np.float64)) * (u @ E)
        s = u.sum(axis=1, keepdims=True)
        u /= s
        tot += np.log(s).mean()
    return tot / (n - 1)


def make_consts(Ep_bf16, end):
    CW = 96 + 2 + 1
    consts = np.zeros((P2, CW), ml_dtypes.bfloat16)
    consts[:T, :T] = Ep_bf16                   # half-A forward block
    consts[T:, T:2 * T] = Ep_bf16              # half-B forward block
    consts[:T, 96] = 1.0                       # lhsT_sum col 0: half-A sum
    consts[T:, 97] = 1.0                       # lhsT_sum col 1: half-B sum
    consts[T:, 98] = np.exp(end.astype(np.float64)).astype(ml_dtypes.bfloat16)
    return consts


def stage_inputs(emissions, start, end, trans):
    """Host-side restaging: per-core [P2, N, 512] bf16 exp(emission) tiles."""
    c0 = _calibrate_c0(emissions, start, trans)
    Ep = (np.exp(trans.astype(np.float64)) * np.exp(-c0)).astype(ml_dtypes.bfloat16)
    consts = make_consts(Ep, end)

    in_maps = []
    for core in range(N_CORES):
        sl = slice(core * BL, (core + 1) * BL)
        em = emissions[sl].astype(np.float32).copy()      # [BL, S, T]
        em[:, 0, :] += start.astype(np.float32)
        w = np.exp(em).astype(ml_dtypes.bfloat16)          # [BL, S, T]
        stg = np.empty((P2, N, NG * GF), ml_dtypes.bfloat16)
        for c in range(NCH):
            g, cg = divmod(c, GW)
            colsl = slice(g * GF + cg * BL, g * GF + (cg + 1) * BL)
            sA = L * (2 * c)        # segment 2c   (0-indexed) starts at L*j
            sB = L * (2 * c + 1)    # segment 2c+1
            stg[:T, :, colsl] = w[:, sA:sA + N, :].transpose(2, 1, 0)
            stg[T:, :, colsl] = w[:, sB:sB + N, :].transpose(2, 1, 0)
        in_maps.append({"wstg": stg, "consts": consts})
    return in_maps, c0


def unpack_logZ(sacc, zfin, c0):
    """Recover logZ[BL] from device outputs of one core (float64 host math)."""
    lsac = np.log(sacc.astype(np.float64))     # [2, NG*2*GF]
    lz = np.log(zfin.astype(np.float64))[0]    # [BL]
    # true-log of record at local step k needs +k*c0 (E is exp(-c0)-prescaled)
    Lrec = np.empty((2, NCH, len(K_REC), BL))  # [half, chain, rec, batch]
    for c in range(NCH):
        g, cg = divmod(c, GW)
        for ri, k in enumerate(K_REC):
            col = (g * len(K_REC) + ri) * GF + cg * BL
            Lrec[:, c, ri] = lsac[:, col:col + BL] + k * c0
    # segment j = 2c + h;  j=0 exact from t=0, j=R-1 ends with the f-dot
    Lst = np.empty((R, BL))
    Len = np.empty((R, BL))
    for c in range(NCH):
        for h in range(2):
            Lst[2 * c + h] = Lrec[h, c, 0]
            Len[2 * c + h] = Lrec[h, c, 1]
    logZ = Len[0].copy()
    for j in range(1, R - 1):
        logZ += Len[j] - Lst[j]
    logZ += (lz + (N - 1) * c0) - Lst[R - 1]
    return logZ


def _device_logZ(emissions, start, end, trans):
    global LAST_RESULTS
    nc = _get_program()
    in_maps, c0 = stage_inputs(emissions, start, end, trans)
    res = run_bass_kernel_spmd(
        nc, in_maps, core_ids=list(range(N_CORES)), trace=TRACE,
    )
    LAST_RESULTS = res
    logZ = np.empty(B, np.float32)
    for core in range(N_CORES):
        r = res.results[core]
        logZ[core * BL:(core + 1) * BL] = unpack_logZ(
            np.asarray(r["sacc"]), np.asarray(r["zfin"]), c0).astype(np.float32)
    return logZ


def _numpy_fallback(emissions, mask, start, end, trans):
    """Faithful float64 reference implementation (handles any mask)."""
    def fwd(use_mask):
        a = start[None, :].astype(np.float64) + emissions[:, 0].astype(np.float64)
        tr = trans.astype(np.float64)
        for t in range(1, emissions.shape[1]):
            inner = a[:, :, None] + tr[None] + emissions[:, t].astype(np.float64)[:, None, :]
            m = inner.max(axis=1, keepdims=True)
            new = np.log(np.exp(inner - m).sum(axis=1)) + m[:, 0, :]
            if use_mask:
                a = np.where(mask[:, t][:, None], new, a)
            else:
                a = new
        fin = a + end[None].astype(np.float64)
        m = fin.max(axis=1, keepdims=True)
        return np.log(np.exp(fin - m).sum(axis=1)) + m[:, 0]

    score = fwd(True)
    partition = fwd(False)
    return (partition - score).astype(np.float32)


def kernel(emissions, mask, start_transitions, end_transitions, transitions):
    emissions = np.asarray(emissions, dtype=np.float32)
    mask = np.asarray(mask)
    start = np.asarray(start_transitions, dtype=np.float32)
    end = np.asarray(end_transitions, dtype=np.float32)
    trans = np.asarray(transitions, dtype=np.float32)

    if not mask.all():
        return _numpy_fallback(emissions, mask, start, end, trans)

    # With an all-ones mask the masked recursion's where(mask, new, old) is
    # the identity, so score == partition; both come from the same forward
    # pass, computed on the 8 NeuronCores.
    logZ = _device_logZ(emissions, start, end, trans)
    partition = logZ
    score = logZ
    return (partition - score).astype(np.float32)
